# revision 17
# baseline (speedup 1.0000x reference)
"""Trainium2 Bass kernel: 3x3 valid 2D cross-correlation on an 8192x8192 f32 image.

Strategy (8 NeuronCores, pure spatial/data parallel, fp16 I/O):
  - The rel-err budget (2e-2) admits fp16 end-to-end (~3.5e-4 actual), which
    halves HBM traffic vs f32: per core ~17.1 MB read + ~16.8 MB write at
    ~400 GB/s/core => ~85 us DMA floor (the kernel is HBM-bandwidth-bound,
    with the TensorEngine nearly co-critical at ~92 us).
  - Column-shard on the host: core i receives the fp16 input column slab
    [8192, 1026] = cols [1024*i, 1024*i + 1026) (2-col halo; right edge
    zero-padded, garbage output cols discarded on host).
  - Per core, 65 identical row tiles: [128 in rows -> 126 out rows] x 1024
    out cols (65*126 = 8190 exactly; tile t reads rows [126t, 126t+128)).
    Per tile, 2 PSUM chunks of 512 cols; per chunk 3 TensorEngine matmuls
    (fp16 operands, 1 col/cycle) accumulate:
        out[y, c] = sum_dx (M_dx.T @ X)[y, c+dx]
    where M_dx[k, y] = w[k-y, dx] is a 3-diagonal band matrix built on the
    host from the 3x3 weight.
  - DMA-instruction issue is expensive (~1.2 us seq+HWDGE per instruction),
    so transfers are batched via custom 3-D DRAM access patterns: loads move
    LK=4 tiles per instruction (the 2-row inter-tile halo makes the read AP
    overlap, which is legal), stores move SK=2 tiles. All x loads and all
    y stores issue on the SP ring; the m load goes on the ACT ring; ACT/DVE
    only do PSUM->SBUF copies (f32->fp16, even/odd chunks).
  - DMA completions can retire out of order across the 16 DMA engines, so
    every buffer slot gets its own semaphore (a single counting semaphore
    cannot prove a *specific* transfer finished).
"""

import numpy as np

import concourse.bass as bass
import concourse.mybir as mybir
from concourse.bass_utils import run_bass_kernel_spmd

H = W = 8192
KH = KW = 3
N_CORES = 8
OUT_H = H - KH + 1  # 8190
OUT_W = W - KW + 1  # 8190

COLS_PER_CORE = 1024          # output cols per core (core 7: keep 1022)
IN_COLS = COLS_PER_CORE + KW - 1  # 1026
TILE_OUT = 126                # output rows per 128-partition input tile
TILE_IN = TILE_OUT + KH - 1   # 128
N_TILES = OUT_H // TILE_OUT   # 65 (exact)
CHUNK = 512                   # PSUM bank width (fp32)
N_CHUNKS = COLS_PER_CORE // CHUNK  # 2

XBUFS = 16                    # xb tile slots (slot = t % 16)
XSEMS = 8                     # load-completion semaphores (round-robin)
# Load groups (t0, nt): small at the start so PE can begin ~2us in and the
# DMA power-ramp stragglers delay less work; steady-state 4-tile groups to
# amortize the ~1.2us per-instruction issue cost (seq + HWDGE).
LOADS = [(0, 1), (1, 1), (2, 2), (4, 2), (6, 2)] + [
    (t0, min(4, N_TILES - t0)) for t0 in range(8, N_TILES, 4)
]
SK = 2                        # tiles per store batch
OG = 8                        # ob batch slots (16 tiles of cushion)
OBUFS = OG * SK               # 16 tile slots
N_SB = (N_TILES - 1) // SK    # 32 full batches (tiles 0..63); tile 64 is
                              # stored whole from the ACT ring at the end

_NC_CACHE = {}


def _build_program():
    nc = bass.Bass("TRN2", target_bir_lowering=False, debug=False)
    x = nc.declare_dram_parameter(
        "x", [H, IN_COLS], mybir.dt.float16, isOutput=False
    )
    m = nc.declare_dram_parameter(
        "m", [128, KW * TILE_OUT], mybir.dt.float16, isOutput=False
    )
    y = nc.declare_dram_parameter(
        "y", [OUT_H, COLS_PER_CORE], mybir.dt.float16, isOutput=True
    )

    xb = nc.alloc_sbuf_tensor(
        "xb", [128, XBUFS * IN_COLS], mybir.dt.float16).ap()
    ob = nc.alloc_sbuf_tensor(
        "ob", [128, OBUFS * COLS_PER_CORE], mybir.dt.float16).ap()
    mt = nc.alloc_sbuf_tensor("mt", [128, KW * TILE_OUT], mybir.dt.float16).ap()
    pb = [nc.alloc_psum_tensor(f"pb{i}", [128, CHUNK], mybir.dt.float32).ap()
          for i in range(8)]

    sm = nc.alloc_semaphore("sm")
    sxl = [nc.alloc_semaphore(f"sxl{s}") for s in range(XSEMS)]
    sob = [nc.alloc_semaphore(f"sob{o}") for o in range(OG)]
    s_mm = nc.alloc_semaphore("s_mm")
    s_cpA = nc.alloc_semaphore("s_cpA")
    s_cpD = nc.alloc_semaphore("s_cpD")

    # tile -> index of the load group that brings it in
    tile_load = {}
    for li, (t0, nt) in enumerate(LOADS):
        for t in range(t0, t0 + nt):
            tile_load[t] = li

    def load_group_aps(t0, nt):
        r0 = t0 * TILE_OUT
        in_ap = x[r0:r0 + TILE_IN, :].unsqueeze(1)
        in_ap.ap = mybir.VecI64Pair(
            [[IN_COLS, TILE_IN], [TILE_OUT * IN_COLS, nt], [1, IN_COLS]]
        )
        cb = (t0 % XBUFS) * IN_COLS
        out_ap = xb[:TILE_IN, cb:cb + nt * IN_COLS]
        return out_ap, in_ap

    def emit_store(eng, b):
        eng.wait_ge(s_cpA, SK * b + SK)
        eng.wait_ge(s_cpD, SK * b + SK)
        r0 = SK * b * TILE_OUT
        out_ap = y[r0:r0 + TILE_OUT, :].unsqueeze(1)
        out_ap.ap = mybir.VecI64Pair(
            [[COLS_PER_CORE, TILE_OUT],
             [TILE_OUT * COLS_PER_CORE, SK],
             [1, COLS_PER_CORE]]
        )
        cb = ((SK * b) % OBUFS) * COLS_PER_CORE
        in_ap = ob[:TILE_OUT, cb:cb + SK * COLS_PER_CORE]
        eng.dma_start(out=out_ap, in_=in_ap).then_inc(sob[b % OG], 16)

    with nc.Block() as block:

        # Stores alternate rings so the DMA engines are fed from two issue
        # streams (a single ~96%-busy SP ring caps sustained bandwidth):
        # odd batches on SP, even batches on ACT (emitted right after
        # ACT's own copy of the batch's second tile), tile 64 on ACT.
        # SP ring: all x loads + odd store batches. An SP store batch is
        # emitted once its tiles are >= 14 behind the load head, so
        # ring-order readiness stays monotone (a tighter interleave would
        # head-of-line block the loads and collapse the prefetch pipeline).
        @block.sync
        def _(sync):
            b_next = 0
            for li, (t0, nt) in enumerate(LOADS):
                if t0 >= XBUFS:
                    # xb slot reuse: previous occupants fully consumed
                    sync.wait_ge(s_mm, N_CHUNKS * (t0 - XBUFS + nt))
                out_ap, in_ap = load_group_aps(t0, nt)
                sync.dma_start(out=out_ap, in_=in_ap).then_inc(
                    sxl[li % XSEMS], 16)
                while b_next < N_SB and SK * b_next + 1 <= t0 + nt - 14:
                    if b_next % 2 == 1:
                        emit_store(sync, b_next)
                    b_next += 1
            while b_next < N_SB:
                if b_next % 2 == 1:
                    emit_store(sync, b_next)
                b_next += 1
            # tile 64 is stored whole on the ACT ring (see scalar block):
            # one DMA issue (~1.4 us engine hold each) in parallel with
            # SP's last batch beats serializing extra issues here
            for o in range(OG):
                n = len(range(o, N_SB, OG)) * 16 + (16 if o == 0 else 0)
                sync.wait_ge(sob[o], n)

        @block.tensor
        def _(tensor):
            tensor.wait_ge(sm, 16)
            for t in range(N_TILES):
                li = tile_load[t]
                if t == LOADS[li][0]:
                    tensor.wait_ge(sxl[li % XSEMS], 16 * (li // XSEMS + 1))
                xcb = (t % XBUFS) * IN_COLS
                for ci in range(N_CHUNKS):
                    gc = t * N_CHUNKS + ci
                    if gc >= 8:
                        # PSUM bank free once chunk gc-8's copy retired;
                        # chunk parity == bank parity (8 banks, 2 chunks)
                        if ci == 0:
                            tensor.wait_ge(s_cpA, t - 3)
                        else:
                            tensor.wait_ge(s_cpD, t - 3)
                    c0 = ci * CHUNK
                    for dx in range(KW):
                        ins = nc.tensor.matmul(
                            pb[gc % 8][:TILE_OUT, :CHUNK],
                            mt[:TILE_IN, dx * TILE_OUT:(dx + 1) * TILE_OUT],
                            xb[:TILE_IN, xcb + c0 + dx:xcb + c0 + dx + CHUNK],
                            start=(dx == 0),
                            stop=(dx == KW - 1),
                        )
                        if dx == KW - 1:
                            ins.then_inc(s_mm, 1)

        @block.scalar
        def _(scalar):
            scalar.dma_start(out=mt, in_=m[:]).then_inc(sm, 16)
            for t in range(N_TILES):
                bt = t // SK
                if bt >= OG:
                    # ob batch slot reuse: batch bt-OG's store retired
                    scalar.wait_ge(sob[bt % OG], 16 * (bt // OG))
                scalar.wait_ge(s_mm, N_CHUNKS * t + 1)
                oc = (t % OBUFS) * COLS_PER_CORE
                nc.scalar.copy(
                    out=ob[:TILE_OUT, oc:oc + CHUNK],
                    in_=pb[(N_CHUNKS * t) % 8][:TILE_OUT, :CHUNK],
                ).then_inc(s_cpA, 1)
                # even store batches issue here: ACT's own copy of tile
                # t=2b+1 just retired in-stream, only DVE's copies need a
                # wait (inside emit_store)
                if t % 2 == 1 and (t // 2) % 2 == 0:
                    emit_store(scalar, t // 2)
            # store tile 64 whole from the ACT ring: chunk0's copy just
            # retired in-stream, only DVE's chunk1 copy needs a wait
            tl = N_TILES - 1
            r0 = tl * TILE_OUT
            oc = (tl % OBUFS) * COLS_PER_CORE
            scalar.wait_ge(s_cpD, tl + 1)
            scalar.dma_start(
                out=y[r0:r0 + TILE_OUT, :],
                in_=ob[:TILE_OUT, oc:oc + COLS_PER_CORE],
            ).then_inc(sob[0], 16)

        @block.vector
        def _(vector):
            for t in range(N_TILES):
                bt = t // SK
                if bt >= OG:
                    vector.wait_ge(sob[bt % OG], 16 * (bt // OG))
                vector.wait_ge(s_mm, N_CHUNKS * t + 2)
                oc = (t % OBUFS) * COLS_PER_CORE
                nc.vector.tensor_copy(
                    out=ob[:TILE_OUT, oc + CHUNK:oc + COLS_PER_CORE],
                    in_=pb[(N_CHUNKS * t + 1) % 8][:TILE_OUT, :CHUNK],
                ).then_inc(s_cpD, 1)

    return nc


def _get_program():
    if "nc" not in _NC_CACHE:
        _NC_CACHE["nc"] = _build_program()
    return _NC_CACHE["nc"]


def _band_matrices(weight: np.ndarray) -> np.ndarray:
    """m[k, dx*126 + y] = w[k-y, dx] for 0 <= k-y < 3."""
    mm = np.zeros((128, KW * TILE_OUT), dtype=np.float32)
    for dx in range(KW):
        for dy in range(KH):
            ys = np.arange(TILE_OUT)
            mm[ys + dy, dx * TILE_OUT + ys] = weight[dy, dx]
    return mm.astype(np.float16)


def _in_maps(x, weight):
    mmat = _band_matrices(weight)
    xh = np.asarray(x, dtype=np.float32).astype(np.float16)
    maps = []
    for i in range(N_CORES):
        c0 = i * COLS_PER_CORE
        c1 = min(c0 + IN_COLS, W)
        slab = np.zeros((H, IN_COLS), dtype=np.float16)
        slab[:, : c1 - c0] = xh[:, c0:c1]
        maps.append({"x": np.ascontiguousarray(slab), "m": mmat})
    return maps


def kernel(x: np.ndarray, weight: np.ndarray) -> np.ndarray:
    x = np.asarray(x, dtype=np.float32)
    weight = np.asarray(weight, dtype=np.float32)
    assert x.shape == (H, W) and weight.shape == (KH, KW)

    nc = _get_program()
    res = run_bass_kernel_spmd(nc, _in_maps(x, weight),
                               core_ids=list(range(N_CORES)))

    out = np.empty((OUT_H, OUT_W), dtype=np.float32)
    for i in range(N_CORES):
        c0 = i * COLS_PER_CORE
        keep = min(COLS_PER_CORE, OUT_W - c0)
        out[:, c0:c0 + keep] = res.results[i]["y"][:, :keep].astype(np.float32)
    return out
